# revision 2
# baseline (speedup 1.0000x reference)
"""MinGRU (2-layer bidirectional) Bass kernel for Trainium2, 8 NeuronCores.

Strategy: data-parallel over batch (B=8 -> 1 batch element per core).
Per core the recurrence h_t = a_t*h_{t-1} + b_t (diagonal per channel) runs on
the Vector engine via tensor_tensor_scan with channels on partitions and time
on the free axis; matmuls run in float32r (full-rate fp32) on the PE; the
backward direction uses negative-stride access patterns so nothing is ever
physically reversed.
"""
import os
import numpy as np
from contextlib import ExitStack

import concourse.bacc as bacc
import concourse.tile as tile
from concourse import mybir
from concourse import bass_utils

F32 = mybir.dt.float32
F32R = mybir.dt.float32r
SIG = mybir.ActivationFunctionType.Sigmoid
MULT = mybir.AluOpType.mult
ADD = mybir.AluOpType.add
MAX = mybir.AluOpType.max

T = 8192
D = 256
H = 256
BLK = 512
NBLK = T // BLK  # 16


def _build():
    nc = bacc.Bacc("TRN2", target_bir_lowering=False, debug=False)

    xs_d = nc.dram_tensor("xs", [T, D], F32, kind="ExternalInput").ap()
    w0f_d = nc.dram_tensor("w0f", [2 * H, D], F32, kind="ExternalInput").ap()
    w0b_d = nc.dram_tensor("w0b", [2 * H, D], F32, kind="ExternalInput").ap()
    w1f_d = nc.dram_tensor("w1f", [2 * H, 2 * H], F32, kind="ExternalInput").ap()
    w1b_d = nc.dram_tensor("w1b", [2 * H, 2 * H], F32, kind="ExternalInput").ap()
    ident_d = nc.dram_tensor("ident", [128, 128], F32, kind="ExternalInput").ap()

    out_d = nc.dram_tensor("out", [T, 2 * H], F32, kind="ExternalOutput").ap()
    hlast_d = nc.dram_tensor("hlast", [4, H], F32, kind="ExternalOutput").ap()

    with tile.TileContext(nc) as tc, ExitStack() as ctx:
        persist = ctx.enter_context(tc.tile_pool(name="persist", bufs=1))
        dram = ctx.enter_context(tc.tile_pool(name="dram", bufs=1, space="DRAM"))

        ident = persist.tile([128, 128], F32, tag="ident")
        nc.sync.dma_start(ident[:], ident_d[:])

        # ---- weights: load, PE-transpose, round to f32r ----
        # W [2H, K] in DRAM -> list of K/128 lhsT tiles [128k, 512m] (f32r)
        def prep_w(w_d, K, name, wst, wps):
            kt = K // 128
            outs = [persist.tile([128, 512], F32R, tag=f"{name}_{kk}", name=f"{name}_{kk}") for kk in range(kt)]
            stages = []
            for m in range(4):
                st = wst.tile([128, K], F32, tag=f"wst{m}")
                nc.sync.dma_start(st[:], w_d[m * 128:(m + 1) * 128, :])
                stages.append(st)
            for kk in range(kt):
                ps = wps.tile([128, 512], F32, tag="wps")
                for m in range(4):
                    nc.tensor.transpose(
                        ps[:, m * 128:(m + 1) * 128],
                        stages[m][:, kk * 128:(kk + 1) * 128],
                        ident[:],
                    )
                nc.scalar.copy(outs[kk][:], ps[:])
            return outs

        with tc.tile_pool(name="wst", bufs=2) as wst, \
             tc.tile_pool(name="wps", bufs=2, space="PSUM") as wps:
            w0fT = prep_w(w0f_d, D, "w0f", wst, wps)
            w0bT = prep_w(w0b_d, D, "w0b", wst, wps)
            w1fT = prep_w(w1f_d, 2 * H, "w1f", wst, wps)
            w1bT = prep_w(w1b_d, 2 * H, "w1b", wst, wps)

        # persistent layer-0 forward hidden states (f32r, rhs of layer-1 matmuls)
        h0f = [persist.tile([128, T], F32R, tag=f"h0f{p}", name=f"h0f{p}") for p in range(2)]
        # layer-0 backward states spilled to DRAM scratch
        h0b_scr = [dram.tile([128, T], F32R, tag=f"h0bscr{p}", name=f"h0bscr{p}") for p in range(2)]

        # activation pipeline: from hg psum tiles (h part + gate part) build
        # a = sigmoid(-g), b = sigmoid(g) * max(h + 0.5, sigmoid(h))
        def act_block(hgh, hgg, half, ab, sc):
            a_t = ab.tile([128, BLK], F32, tag=f"a{half}")
            b_t = ab.tile([128, BLK], F32, tag=f"b{half}")
            u_t = sc.tile([128, BLK], F32, tag=f"u{half}")
            nc.scalar.activation(b_t[:], hgg[:], SIG)                  # s = sigmoid(g)
            nc.vector.tensor_scalar(a_t[:], b_t[:], -1.0, 1.0, MULT, ADD)  # a = 1 - s
            nc.scalar.activation(u_t[:], hgh[:], SIG)                  # u = sigmoid(h)
            nc.vector.scalar_tensor_tensor(u_t[:], hgh[:], 0.5, u_t[:], ADD, MAX)
            nc.vector.tensor_tensor(b_t[:], b_t[:], u_t[:], MULT)      # b = s * m
            return a_t, b_t

        # ---------------- layer 0 ----------------
        with tc.tile_pool(name="xstage", bufs=4) as xstage, \
             tc.tile_pool(name="xtp", bufs=2, space="PSUM") as xtp, \
             tc.tile_pool(name="hgp", bufs=2, space="PSUM") as hgp, \
             tc.tile_pool(name="ab", bufs=2) as ab, \
             tc.tile_pool(name="sc", bufs=2) as sc, \
             tc.tile_pool(name="hb", bufs=3) as hbp:

            xT = [persist.tile([128, T], F32R, tag=f"xT{p}", name=f"xT{p}") for p in range(2)]

            # forward direction: stream+transpose x, matmul, scan ascending
            for k in range(NBLK):
                stages = []
                for i in range(4):
                    stg = xstage.tile([128, D], F32, tag=f"xs{i}")
                    nc.sync.dma_start(stg[:], xs_d[k * BLK + i * 128: k * BLK + (i + 1) * 128, :])
                    stages.append(stg)
                for p in range(2):
                    ps = xtp.tile([128, BLK], F32, tag=f"xtp{p}")
                    for i in range(4):
                        nc.tensor.transpose(
                            ps[:, i * 128:(i + 1) * 128],
                            stages[i][:, p * 128:(p + 1) * 128],
                            ident[:],
                        )
                    nc.scalar.copy(xT[p][:, k * BLK:(k + 1) * BLK], ps[:])
                for half in range(2):
                    hgh = hgp.tile([128, BLK], F32, tag="hgh")
                    hgg = hgp.tile([128, BLK], F32, tag="hgg")
                    for kk in range(2):
                        rhs = xT[kk][:, k * BLK:(k + 1) * BLK]
                        nc.tensor.matmul(hgh[:], w0fT[kk][:, half * 128:(half + 1) * 128],
                                         rhs, start=(kk == 0), stop=(kk == 1))
                    for kk in range(2):
                        rhs = xT[kk][:, k * BLK:(k + 1) * BLK]
                        nc.tensor.matmul(hgg[:], w0fT[kk][:, (half + 2) * 128:(half + 3) * 128],
                                         rhs, start=(kk == 0), stop=(kk == 1))
                    a_t, b_t = act_block(hgh, hgg, half, ab, sc)
                    init = 0.0 if k == 0 else h0f[half][:, k * BLK - 1: k * BLK]
                    nc.vector.tensor_tensor_scan(
                        h0f[half][:, k * BLK:(k + 1) * BLK], a_t[:], b_t[:], init, MULT, ADD)
            for half in range(2):
                nc.sync.dma_start(
                    hlast_d[0:1, half * 128:(half + 1) * 128].rearrange("a b -> b a"),
                    h0f[half][:, T - 1: T].bitcast(F32))

            # backward direction: blocks descending, scans through reversed APs
            hb_prev = [None, None]
            for k in range(NBLK - 1, -1, -1):
                for half in range(2):
                    hgh = hgp.tile([128, BLK], F32, tag="hgh")
                    hgg = hgp.tile([128, BLK], F32, tag="hgg")
                    for kk in range(2):
                        rhs = xT[kk][:, k * BLK:(k + 1) * BLK]
                        nc.tensor.matmul(hgh[:], w0bT[kk][:, half * 128:(half + 1) * 128],
                                         rhs, start=(kk == 0), stop=(kk == 1))
                    for kk in range(2):
                        rhs = xT[kk][:, k * BLK:(k + 1) * BLK]
                        nc.tensor.matmul(hgg[:], w0bT[kk][:, (half + 2) * 128:(half + 3) * 128],
                                         rhs, start=(kk == 0), stop=(kk == 1))
                    a_t, b_t = act_block(hgh, hgg, half, ab, sc)
                    hb = hbp.tile([128, BLK], F32R, tag=f"hb{half}")
                    init = 0.0 if k == NBLK - 1 else hb_prev[half][:, 0:1]
                    nc.vector.tensor_tensor_scan(
                        hb[:, ::-1], a_t[:, ::-1], b_t[:, ::-1], init, MULT, ADD)
                    hb_prev[half] = hb
                    nc.sync.dma_start(h0b_scr[half][:, k * BLK:(k + 1) * BLK], hb[:])
                    if k == 0:
                        nc.sync.dma_start(
                            hlast_d[1:2, half * 128:(half + 1) * 128].rearrange("a b -> b a"),
                            hb[:, 0:1].bitcast(F32))

        # ---------------- layer 1 ----------------
        with tc.tile_pool(name="rb", bufs=3) as rbp, \
             tc.tile_pool(name="hg1", bufs=2, space="PSUM") as hg1p, \
             tc.tile_pool(name="otp", bufs=3, space="PSUM") as otpp, \
             tc.tile_pool(name="ab1", bufs=2) as ab1, \
             tc.tile_pool(name="sc1", bufs=2) as sc1, \
             tc.tile_pool(name="h1", bufs=3) as h1p, \
             tc.tile_pool(name="ost", bufs=4) as ostp:

            def l1_dir(wT, reverse):
                hoff = H if reverse else 0
                hl_row = 3 if reverse else 2
                h1_prev = [None, None]
                ks = range(NBLK - 1, -1, -1) if reverse else range(NBLK)
                for k in ks:
                    rbs = []
                    for p in range(2):
                        rb = rbp.tile([128, BLK], F32R, tag=f"rb{p}")
                        nc.sync.dma_start(rb[:], h0b_scr[p][:, k * BLK:(k + 1) * BLK])
                        rbs.append(rb)
                    h1s = []
                    for half in range(2):
                        hgh = hg1p.tile([128, BLK], F32, tag="hgh1")
                        hgg = hg1p.tile([128, BLK], F32, tag="hgg1")
                        for kk in range(4):
                            rhs = (h0f[kk][:, k * BLK:(k + 1) * BLK] if kk < 2
                                   else rbs[kk - 2][:])
                            nc.tensor.matmul(hgh[:], wT[kk][:, half * 128:(half + 1) * 128],
                                             rhs, start=(kk == 0), stop=(kk == 3))
                        for kk in range(4):
                            rhs = (h0f[kk][:, k * BLK:(k + 1) * BLK] if kk < 2
                                   else rbs[kk - 2][:])
                            nc.tensor.matmul(hgg[:], wT[kk][:, (half + 2) * 128:(half + 3) * 128],
                                             rhs, start=(kk == 0), stop=(kk == 3))
                        a_t, b_t = act_block(hgh, hgg, half, ab1, sc1)
                        h1 = h1p.tile([128, BLK], F32, tag=f"h1_{half}")
                        if reverse:
                            init = 0.0 if k == NBLK - 1 else h1_prev[half][:, 0:1]
                            nc.vector.tensor_tensor_scan(
                                h1[:, ::-1], a_t[:, ::-1], b_t[:, ::-1], init, MULT, ADD)
                        else:
                            init = 0.0 if k == 0 else h1_prev[half][:, BLK - 1: BLK]
                            nc.vector.tensor_tensor_scan(h1[:], a_t[:], b_t[:], init, MULT, ADD)
                        h1_prev[half] = h1
                        h1s.append(h1)
                        if (not reverse and k == NBLK - 1) or (reverse and k == 0):
                            col = 0 if reverse else BLK - 1
                            nc.sync.dma_start(
                                hlast_d[hl_row:hl_row + 1, half * 128:(half + 1) * 128]
                                .rearrange("a b -> b a"),
                                h1[:, col:col + 1])
                    # transpose [h, t] -> [t, h] and store this block's half-rows
                    for i in range(4):
                        otp = otpp.tile([128, H], F32, tag="otp")
                        for half in range(2):
                            nc.tensor.transpose(
                                otp[:, half * 128:(half + 1) * 128],
                                h1s[half][:, i * 128:(i + 1) * 128],
                                ident[:],
                            )
                        ost = ostp.tile([128, H], F32, tag="ost")
                        nc.scalar.copy(ost[:], otp[:])
                        nc.sync.dma_start(
                            out_d[k * BLK + i * 128: k * BLK + (i + 1) * 128,
                                  hoff:hoff + H],
                            ost[:])

            l1_dir(w1fT, reverse=False)
            l1_dir(w1bT, reverse=True)

    nc.compile()
    return nc


_NC = None
last_results = None


def kernel(x, W0f, W0b, W1f, W1b):
    global _NC, last_results
    if _NC is None:
        _NC = _build()
    B = x.shape[0]
    ident = np.eye(128, dtype=np.float32)
    in_maps = []
    for b in range(B):
        in_maps.append(dict(
            xs=np.ascontiguousarray(x[b], np.float32),
            w0f=np.ascontiguousarray(W0f, np.float32),
            w0b=np.ascontiguousarray(W0b, np.float32),
            w1f=np.ascontiguousarray(W1f, np.float32),
            w1b=np.ascontiguousarray(W1b, np.float32),
            ident=ident,
        ))
    res = bass_utils.run_bass_kernel_spmd(_NC, in_maps, list(range(B)))
    last_results = res
    out = np.stack([res.results[b]["out"] for b in range(B)])        # [B, T, 2H]
    hl = np.stack([res.results[b]["hlast"] for b in range(B)], 1)    # [4, B, H]
    return out, hl
